# revision 37
# baseline (speedup 1.0000x reference)
"""Trainium2 Bass kernel for a ViT-style EncoderBlock (v3).

Problem: B=4, N=2048, D=768, H=12 heads (hd=64), FFN hidden 3072, fp32.
  y = x + proj(attn(LN1(x))) ;  out = y + fc2(gelu(fc1(LN2(y))))

Sharding (8 cores, zero communication): core c handles batch b=c//2 and
query-half s=c%2 (1024 query rows).  Each core receives the full batch-b
sequence (2048 rows) with its own query rows permuted to the front, computes
K/V over all 2048 rows, attention/FFN for its 1024 rows, and returns its
[1024, 768] slice of the output.  Host reassembles.

v3 design notes (evolved from v2 = 546 us after trace analysis):
 - Phase A fuses LN1 and the V projection per 128-row x tile so the PE works
   through the LN phase instead of idling behind DVE stats; identity warm-up
   matmuls (some staggered on the x-tile DMAs) keep the HAM clock gate open.
 - Input DMAs split across rings: consts + QKV weights (per-d-column) on
   sync, x tiles + proj/fc1 weights on gpsimd; fc2 weights stream in 4
   chunks on both rings during the tail, ahead of the (reordered-last) fc2.
 - LN scale/bias applies alternate ACT / DVE (both read PSUM).
 - Attention runs ch(=query 512-chunk)-outer / head-pair-inner: after ch0,
   OT2[:, :, 0:512] is complete, so proj + residual + LN2 + 12 of 24 fc1
   chains for chunk 0 hide inside ch1's ACT-bound (softmax exp) window.
   All K/Q projection chains run inside ch0.
 - Softmax denominators: reciprocal_approx_fast (~5x cheaper on DVE); the
   two denominator broadcasts are column-tiled into one PSUM tile and the
   normalize multiply covers both heads at once.
 - LN2 uses a DVE-only Newton rsqrt so no ACT Sqrt table swap disturbs the
   Exp table during attention; ch1's fc1 chains defer their gelu (bias via
   DVE) until after attention (Gelu table loads once, in the tail).
 - Softmax skips max-subtraction (scores ~N(0,1) after 1/8 scale).

SBUF lifetime plan (stack allocator, LIFO per side):
  left:  consts, pjw, w1a, OT2, hT, wk, wq, wv, xball | phase-A pools
         (xball+wv released after phase A; hT/wk/wq after attention ch0;
          then res1/x2T/h1r; w2a at tail start)
  right: kT, qT, V4, attention pools (all released after attention),
         then h1a/h1b/h1c + tail pools
"""

import sys

if "/opt/trn_rl_repo" not in sys.path:
    sys.path.insert(0, "/opt/trn_rl_repo")

import numpy as np

B, N, D = 4, 2048, 768
H, HD = 12, 64
HID = 4 * D
NQ = N // 2  # query rows per core
SCALE = HD ** -0.5
EPS = 1e-5

P = 128
DT = D // P          # 6 d-tiles
NQT = NQ // P        # 8 query tiles
NMT = N // P         # 16 kv tiles
HIDT = HID // P      # 24 hidden tiles
VW = HD + 1          # 65: V plus ones column
JH = H // 2          # 6 head pairs
FC1_IN_CH1 = 12      # fc1 chunk-0 chains emitted inside attention ch1

INPUT_NAMES = (
    "ln1_g", "ln1_b", "qkv_w", "proj_w", "proj_b",
    "ln2_g", "ln2_b", "fc1_w", "fc1_b", "fc2_w", "fc2_b",
)


def _encoder_body(tc, out_ap, aps):
    import concourse.bass as bass
    from concourse import mybir
    from concourse.masks import make_identity

    nc = tc.nc
    f32 = mybir.dt.float32
    f32r = mybir.dt.float32r
    bf16 = mybir.dt.bfloat16
    AF = mybir.ActivationFunctionType
    OP = mybir.AluOpType

    def mm(psum, lhsT, rhs, start, stop):
        nc.tensor.matmul(psum, lhsT, rhs, start=start, stop=stop)

    x = aps["x"]

    # ---------------- pools, in LIFO release order ----------------
    consts = tc.alloc_tile_pool(name="consts", bufs=1)
    pjwp = tc.alloc_tile_pool(name="pjwp", bufs=1)
    f1wp = tc.alloc_tile_pool(name="f1w", bufs=1)
    otp = tc.alloc_tile_pool(name="otp", bufs=1)
    hTp = tc.alloc_tile_pool(name="hTp", bufs=1)
    wkp = tc.alloc_tile_pool(name="wkp", bufs=1)
    wqp = tc.alloc_tile_pool(name="wqp", bufs=1)
    wvp = tc.alloc_tile_pool(name="wvp", bufs=1)
    xbp = tc.alloc_tile_pool(name="xbp", bufs=1)
    ktp = tc.alloc_tile_pool(name="ktp", bufs=1, side="right")
    qtp = tc.alloc_tile_pool(name="qtp", bufs=1, side="right")
    v4p = tc.alloc_tile_pool(name="v4p", bufs=1, side="right")

    # ---------------- constants ----------------
    ident = consts.tile([P, P], f32, name="ident")
    make_identity(nc, ident)
    identb = consts.tile([P, P], bf16, name="identb")
    nc.vector.tensor_copy(identb, ident)
    g1c = consts.tile([P, DT], f32, name="g1c")
    b1c = consts.tile([P, DT], f32, name="b1c")
    g2c = consts.tile([P, DT], f32, name="g2c")
    b2c = consts.tile([P, DT], f32, name="b2c")
    f1bc = consts.tile([P, HIDT], f32, name="f1bc")
    pjbf = consts.tile([P, D], f32, name="pjbf")
    ones_f = consts.tile([P, HD], f32, name="ones_f")
    nc.vector.memset(ones_f, 1.0)
    ones64 = consts.tile([1, HD], f32r, name="ones64")
    nc.vector.tensor_copy(ones64, ones_f[0:1, :])

    # weights tiles
    pjw = pjwp.tile([P, JH, D], bf16, name="pjw")
    w1a = f1wp.tile([P, DT, HID], bf16, name="w1a")
    OT2 = otp.tile([P, JH, NQ], bf16, name="OT2")
    hT = hTp.tile([P, DT, N], bf16, name="hT")
    wk = wkp.tile([P, DT, D], bf16, name="wk")
    wq = wqp.tile([P, DT, D], bf16, name="wq")
    wv = wvp.tile([P, DT, D], bf16, name="wv")
    xball = xbp.tile([P, NMT, D], bf16, name="xball")
    kT = ktp.tile([P, DT, N], bf16, name="kT")
    qT = qtp.tile([P, DT, NQ], bf16, name="qT")
    V4 = v4p.tile([P, NMT, H, VW], bf16, name="V4")

    # ---------------- DMA issue (ring order matters) ----------------
    # sync ring (no cast allowed): small f32 consts.
    nc.sync.dma_start(out=g1c, in_=aps["ln1_g"].rearrange("(t p) -> p t", p=P))
    nc.sync.dma_start(out=b1c, in_=aps["ln1_b"].rearrange("(t p) -> p t", p=P))
    nc.sync.dma_start(out=g2c, in_=aps["ln2_g"].rearrange("(t p) -> p t", p=P))
    nc.sync.dma_start(out=b2c, in_=aps["ln2_b"].rearrange("(t p) -> p t", p=P))
    nc.sync.dma_start(out=f1bc, in_=aps["fc1_b"].rearrange("(t p) -> p t", p=P))

    # gpsimd ring (cast f32->bf16): first x tiles + V weights (phase A
    # needs them first), then K/Q weights, then proj + fc1 weights.
    for i in range(3):
        nc.gpsimd.dma_start(out=xball[:, i, :], in_=x[i * P:(i + 1) * P, :])
    for t in range(DT):
        nc.gpsimd.dma_start(
            out=wv[:, t, :], in_=aps["qkv_w"][t * P:(t + 1) * P, 2 * D:3 * D]
        )
    for i in range(3, 10):
        nc.gpsimd.dma_start(out=xball[:, i, :], in_=x[i * P:(i + 1) * P, :])
    for t in range(DT):
        nc.gpsimd.dma_start(
            out=wk[:, t, :], in_=aps["qkv_w"][t * P:(t + 1) * P, D:2 * D]
        )
    for i in range(10, NMT):
        nc.gpsimd.dma_start(out=xball[:, i, :], in_=x[i * P:(i + 1) * P, :])
    for t in range(DT):
        nc.gpsimd.dma_start(
            out=wq[:, t, :], in_=aps["qkv_w"][t * P:(t + 1) * P, 0:D]
        )
    nc.gpsimd.dma_start(out=pjw, in_=aps["proj_w"].rearrange("(j p) d -> p j d", p=P))
    nc.gpsimd.dma_start(out=w1a, in_=aps["fc1_w"].rearrange("(t p) c -> p t c", p=P))
    # pjbf = proj_b + fc2_b broadcast (both residual biases; see v2 notes),
    # built by two DMAs (second accumulates) — no engine op involved.
    # At the end of the gpsimd stream: needed only at the ch1 residuals.
    nc.gpsimd.dma_start(out=pjbf, in_=aps["proj_b"].partition_broadcast(P))
    nc.gpsimd.dma_start(
        out=pjbf, in_=aps["fc2_b"].partition_broadcast(P),
        accum_op=OP.add,
    )

    # ---------------- phase A : fused LN1 + V ----------------
    GRP = 2
    with tc.tile_pool(name="pAwork", bufs=2) as work, \
         tc.tile_pool(name="pAsmall", bufs=2) as small, \
         tc.tile_pool(name="pApsT", bufs=2, space="PSUM") as psT, \
         tc.tile_pool(name="pAboot", bufs=1, space="PSUM") as psBoot, \
         tc.tile_pool(name="pApsV", bufs=2, space="PSUM") as psV:
        # PE warm-up for the HAM clock gate
        wps = psT.tile([P, P], f32, name="warmps", tag="warm", bufs=1)
        for _ in range(48):
            mm(wps, identb, identb, True, True)
        # the V4 softmax-denominator ones column, all tiles in one memset
        nc.vector.memset(V4[:, :, :, HD:VW], 1.0)

        def boot_chain(kind, idx):
            """Bootstrap K/Q head-column-0 chains inside phase A so the
            attention loop starts with its operands ready."""
            ps = psBoot.tile([P, 512], f32, name="boot")
            w_ = wk if kind == "K" else wq
            for t in range(DT):
                mm(ps, w_[:, t, 0:P],
                   hT[:, t, idx * 512:(idx + 1) * 512], t == 0, t == DT - 1)
            if kind == "K":
                nc.vector.tensor_copy(kT[:, 0, idx * 512:(idx + 1) * 512], ps)
            else:
                nc.vector.tensor_copy(qT[:, 0, idx * 512:(idx + 1) * 512], ps)

        boots = {5: ("K", 0), 6: ("Q", 0), 7: ("K", 1)}
        for g in range(NMT // GRP):
            sts, mvs = [], []
            for k in range(GRP):
                i = g * GRP + k
                st = small.tile([P, 2, 6], f32, name=f"st{k}", tag=f"st{k}")
                for h2 in range(2):
                    nc.vector.bn_stats(
                        st[:, h2, :], xball[:, i, h2 * 384:(h2 + 1) * 384]
                    )
                sts.append(st)
            vre = small.tile([P, GRP], f32, name="vre", tag="vre")
            vr = small.tile([P, GRP], f32, name="vr", tag="vr")
            nmr = small.tile([P, GRP], f32, name="nmr", tag="nmr")
            sq = small.tile([P, GRP], f32, name="sq", tag="sq")
            r0 = small.tile([P, GRP], f32, name="r0", tag="r0")
            tt = small.tile([P, GRP], f32, name="tt", tag="tt")
            for k in range(GRP):
                mv = small.tile([P, 2], f32, name=f"mv{k}", tag=f"mv{k}")
                nc.vector.bn_aggr(mv, sts[k])
                mvs.append(mv)
                nc.vector.tensor_scalar_add(vre[:, k:k + 1], mv[:, 1:2], EPS)
            nc.scalar.sqrt(sq, vre)
            nc.vector.reciprocal(r0, sq)
            nc.vector.scalar_tensor_tensor(tt, r0, 1.0, r0, OP.mult, OP.mult)
            nc.vector.tensor_mul(tt, tt, vre)
            nc.vector.tensor_scalar(tt, tt, -0.5, 1.5, OP.mult, OP.add)
            nc.vector.tensor_mul(vr, r0, tt)
            for k in range(GRP):
                nc.vector.tensor_scalar(
                    nmr[:, k:k + 1], mvs[k][:, 0:1], vr[:, k:k + 1], -1.0,
                    OP.mult, OP.mult,
                )
            for k in range(GRP):
                i = g * GRP + k
                h = work.tile([P, D], bf16, name="hln")
                nc.vector.tensor_scalar(
                    h, xball[:, i, :], vr[:, k:k + 1], nmr[:, k:k + 1],
                    OP.mult, OP.add,
                )
                for t in range(DT):
                    ps = psT.tile([P, P], f32, name="trps")
                    mm(ps, h[:, t * P:(t + 1) * P], identb, True, True)
                    if t not in (2, 5):
                        nc.scalar.activation(
                            hT[:, t, i * P:(i + 1) * P], ps, AF.Identity,
                            bias=b1c[:, t:t + 1], scale=g1c[:, t:t + 1],
                        )
                    else:
                        nc.vector.tensor_scalar(
                            hT[:, t, i * P:(i + 1) * P], ps,
                            g1c[:, t:t + 1], b1c[:, t:t + 1], OP.mult, OP.add,
                        )
                psv = psV.tile([P, D], f32, name="vps", padded_shape=[P, 1024])
                for t in range(DT):
                    mm(psv[:, 0:512], hT[:, t, i * P:(i + 1) * P],
                       wv[:, t, 0:512], t == 0, t == DT - 1)
                for t in range(DT):
                    mm(psv[:, 512:768], hT[:, t, i * P:(i + 1) * P],
                       wv[:, t, 512:768], t == 0, t == DT - 1)
                nc.scalar.copy(
                    V4[:, i, 0:8, 0:HD],
                    psv[:, 0:512].rearrange("p (a b) -> p a b", a=8),
                )
                nc.scalar.copy(
                    V4[:, i, 8:12, 0:HD],
                    psv[:, 512:768].rearrange("p (a b) -> p a b", a=4),
                )
            if g in boots:
                boot_chain(*boots[g])
        boot_chain("K", 2)
        boot_chain("K", 3)
    xbp.release()
    wvp.release()

    # ---------------- phase B : attention (ch outer, j inner) -------------
    st_ = {}  # late-bound tiles (res1/x2T/h1r created after ch0)

    with tc.tile_pool(name="a_es", bufs=3, side="right") as esp, \
         tc.tile_pool(name="a_small", bufs=1, side="right") as asmall, \
         tc.tile_pool(name="a_ln2", bufs=2, side="right") as lsm, \
         tc.tile_pool(name="a_xr", bufs=2, side="right") as xrp, \
         tc.tile_pool(name="a_psS", bufs=2, space="PSUM") as psS, \
         tc.tile_pool(name="a_psO", bufs=2, space="PSUM") as psO, \
         tc.tile_pool(name="a_psQ", bufs=2, space="PSUM") as psQ:

        def emit_kq_chain(kind, c, idx):
            ps = psQ.tile([P, 512], f32, name="cq", tag="cq")
            w_ = wk if kind == "K" else wq
            for t in range(DT):
                mm(ps, w_[:, t, c * P:(c + 1) * P],
                   hT[:, t, idx * 512:(idx + 1) * 512], t == 0, t == DT - 1)
            if kind == "K":
                nc.vector.tensor_copy(kT[:, c, idx * 512:(idx + 1) * 512], ps)
            else:
                nc.vector.tensor_copy(qT[:, c, idx * 512:(idx + 1) * 512], ps)

        def emit_norm(stt):
            """Column-tiled broadcast of the two reciprocal denominator rows
            over the 64 head rows each, then one normalize multiply."""
            j, cs, posbP, rdA, rdB = stt
            rbA = psQ.tile([HD, 512], f32, name="rbA", tag="cq")
            mm(rbA, ones64, rdA, True, True)
            nc.vector.tensor_tensor(OT2[0:HD, j, cs], posbP[0:HD, :], rbA, OP.mult)
            rbB = psQ.tile([HD, 512], f32, name="rbB", tag="cq")
            mm(rbB, ones64, rdB, True, True)
            nc.vector.tensor_tensor(OT2[HD:P, j, cs], posbP[HD:P, :], rbB, OP.mult)

        ln2mv = []

        def emit_xr_dma(i):
            xr = xrp.tile([P, D], f32, name="xr", tag="xr")
            nc.sync.dma_start(out=xr, in_=x[i * P:(i + 1) * P, :])
            return xr

        def emit_proj(i, xr):
            isl = slice(i * P, (i + 1) * P)
            psa = psQ.tile([P, 512], f32, name="pja", tag="cq")
            for j in range(JH):
                mm(psa, OT2[:, j, isl], pjw[:, j, 0:512], j == 0, j == JH - 1)
            psb = psQ.tile([P, 512], f32, name="pjb2", tag="cq")
            for j in range(JH):
                mm(psb[:, 0:256], OT2[:, j, isl], pjw[:, j, 512:768],
                   j == 0, j == JH - 1)
            r1 = st_["res1"][:, i, :]
            nc.vector.tensor_add(r1[:, 0:512], psa, xr[:, 0:512])
            nc.vector.tensor_add(r1[:, 512:768], psb[:, 0:256], xr[:, 512:768])
            nc.vector.tensor_tensor(r1, r1, pjbf, OP.add)
            stt = lsm.tile([P, 2, 6], f32, name="l2st", tag=f"l2st{i % 4}")
            for h2 in range(2):
                nc.vector.bn_stats(stt[:, h2, :], r1[:, h2 * 384:(h2 + 1) * 384])
            mv = lsm.tile([P, 2], f32, name="l2mv", tag=f"l2mv{i % 4}")
            nc.vector.bn_aggr(mv, stt)
            ln2mv.append(mv)

        def emit_ln2_batch(base, nt):
            """DVE-only Newton rsqrt for nt tiles (keeps Exp table loaded)."""
            vre = lsm.tile([P, nt], f32, name="l2ve", tag="l2ve")
            vr = lsm.tile([P, nt], f32, name="l2vr", tag="l2vr")
            nmr = lsm.tile([P, nt], f32, name="l2nm", tag="l2nm")
            tt = lsm.tile([P, nt], f32, name="l2tt", tag="l2tt")
            for k in range(nt):
                nc.vector.tensor_scalar_add(
                    vre[:, k:k + 1], ln2mv[base + k][:, 1:2], EPS
                )
            # minimax line seed for 1/sqrt on [0.5, 3.0] + 4 Newton steps
            nc.vector.tensor_scalar(vr, vre, -0.3346, 1.581, OP.mult, OP.add)
            for _ in range(4):
                nc.vector.scalar_tensor_tensor(tt, vr, 1.0, vr, OP.mult, OP.mult)
                nc.vector.tensor_mul(tt, tt, vre)
                nc.vector.tensor_scalar(tt, tt, -0.5, 1.5, OP.mult, OP.add)
                nc.vector.tensor_mul(vr, vr, tt)
            for k in range(nt):
                nc.vector.tensor_scalar(
                    nmr[:, k:k + 1], ln2mv[base + k][:, 0:1], vr[:, k:k + 1],
                    -1.0, OP.mult, OP.mult,
                )
            return vr, nmr

        h2cache = {}

        def emit_trans2(i, vr, nmr, k, tlist):
            if i not in h2cache:
                h2 = lsm.tile([P, D], bf16, name="h2", tag=f"h2{i % 2}")
                nc.vector.tensor_scalar(
                    h2, st_["res1"][:, i, :], vr[:, k:k + 1], nmr[:, k:k + 1],
                    OP.mult, OP.add,
                )
                h2cache[i] = h2
            h2 = h2cache[i]
            for t in tlist:
                ps = psQ.tile([P, P], f32, name="trp2", tag="cq")
                mm(ps, h2[:, t * P:(t + 1) * P], identb, True, True)
                if t % 2 == 0:
                    nc.scalar.activation(
                        st_["x2T"][:, t, i * P:(i + 1) * P], ps, AF.Identity,
                        bias=b2c[:, t:t + 1], scale=g2c[:, t:t + 1],
                    )
                else:
                    nc.vector.tensor_scalar(
                        st_["x2T"][:, t, i * P:(i + 1) * P], ps,
                        g2c[:, t:t + 1], b2c[:, t:t + 1], OP.mult, OP.add,
                    )

        def emit_fc1_deferred(hc):
            psf = psQ.tile([P, 512], f32, name="psf", tag="cq")
            for t in range(DT):
                mm(psf, w1a[:, t, hc * P:(hc + 1) * P],
                   st_["x2T"][:, t, 0:512], t == 0, t == DT - 1)
            nc.vector.tensor_scalar_add(
                st_["h1r"][:, hc, :], psf, f1bc[:, hc:hc + 1]
            )

        # ---------------- attention main loops ----------------
        ln2vn = []
        pending = None
        for ch in range(2):
            cs = slice(ch * 512, (ch + 1) * 512)
            for j in range(JH):
                hA, hB = 2 * j, 2 * j + 1
                kTa, kTb = kT[0:HD, j, :], kT[HD:P, j, :]
                qTa, qTb = qT[0:HD, j, cs], qT[HD:P, j, cs]
                witems = {}
                if ch == 0:
                    if j + 1 < JH:
                        witems = {
                            2: lambda c=j + 1: emit_kq_chain("K", c, 0),
                            4: lambda c=j + 1: emit_kq_chain("K", c, 1),
                            6: lambda c=j + 1: emit_kq_chain("K", c, 2),
                            8: lambda c=j + 1: emit_kq_chain("K", c, 3),
                            13: lambda c=j + 1: emit_kq_chain("Q", c, 0),
                            15: lambda c=j: emit_kq_chain("Q", c, 1),
                        }
                    else:
                        witems = {2: lambda c=j: emit_kq_chain("Q", c, 1)}
                else:
                    if j == 0:
                        # late slots: the chunk-boundary norm flush occupies
                        # the cq ring + DVE early in this loop
                        witems = {
                            8: lambda: emit_proj(0, emit_xr_dma(0)),
                            13: lambda: emit_proj(1, emit_xr_dma(1)),
                        }
                    elif j == 1:
                        witems = {
                            2: lambda: emit_proj(2, emit_xr_dma(2)),
                            6: lambda: emit_proj(3, emit_xr_dma(3)),
                        }
                    elif j == 2:
                        def _ln2_head():
                            ln2vn.append(emit_ln2_batch(0, 4))
                            h2cache.clear()
                            emit_trans2(0, *ln2vn[0], 0, [0, 1, 2])
                        witems = {
                            2: _ln2_head,
                            4: lambda: emit_trans2(0, *ln2vn[0], 0, [3, 4, 5]),
                            6: lambda: emit_trans2(1, *ln2vn[0], 1, [0, 1, 2]),
                            8: lambda: emit_trans2(1, *ln2vn[0], 1, [3, 4, 5]),
                            13: lambda: emit_trans2(2, *ln2vn[0], 2, [0, 1, 2]),
                            15: lambda: emit_trans2(2, *ln2vn[0], 2, [3, 4, 5]),
                        }
                    elif j == 3:
                        witems = {
                            2: lambda: emit_trans2(3, *ln2vn[0], 3, [0, 1, 2]),
                            4: lambda: emit_trans2(3, *ln2vn[0], 3, [3, 4, 5]),
                            6: lambda: emit_fc1_deferred(0),
                            8: lambda: emit_fc1_deferred(1),
                            13: lambda: emit_fc1_deferred(2),
                            15: lambda: emit_fc1_deferred(3),
                        }
                    elif j == 4:
                        witems = {
                            2: lambda: emit_fc1_deferred(4),
                            6: lambda: emit_fc1_deferred(5),
                            8: lambda: emit_fc1_deferred(6),
                            13: lambda: emit_fc1_deferred(7),
                        }
                    elif j == 5:
                        witems = {
                            2: lambda: emit_fc1_deferred(8),
                            6: lambda: emit_fc1_deferred(9),
                            8: lambda: emit_fc1_deferred(10),
                            13: lambda: emit_fc1_deferred(11),
                        }
                poA = psO.tile([VW, 512], f32, name="poA", tag="po")
                poB = psO.tile([VW, 512], f32, name="poB", tag="po")
                # software-pipelined: scores(mt) and exp(mt) issue before
                # attnV(mt-1), so the PE always has scores work in flight
                # while the ACT exp runs
                prev_es = None
                for mt in range(NMT):
                    msl = slice(mt * P, (mt + 1) * P)
                    ps = psS.tile([P, 1024], f32, name="sps")
                    mm(ps[:, 0:512], kTa[:, msl], qTa, True, True)
                    mm(ps[:, 512:1024], kTb[:, msl], qTb, True, True)
                    es = esp.tile([P, 1024], bf16, name="es")
                    nc.scalar.activation(es, ps, AF.Exp, scale=SCALE)
                    if prev_es is not None:
                        mm(poA, V4[:, mt - 1, hA, :], prev_es[:, 0:512],
                           mt - 1 == 0, False)
                        mm(poB, V4[:, mt - 1, hB, :], prev_es[:, 512:1024],
                           mt - 1 == 0, False)
                    prev_es = es
                    if mt == 5 and pending is not None:
                        # previous pair's reciprocals, off the drain path so
                        # they never sit ahead of this pair's PSUM casts
                        _, _, _, denA_, denB_, rdA_, rdB_ = pending
                        with nc.allow_low_precision(reason="softmax recip"):
                            nc.vector.reciprocal(rdA_, denA_)
                            nc.vector.reciprocal(rdB_, denB_)
                    if mt == 11 and pending is not None:
                        emit_norm(pending[:3] + pending[5:])
                        pending = None
                    w = witems.get(mt)
                    if w is not None:
                        w()
                mm(poA, V4[:, NMT - 1, hA, :], prev_es[:, 0:512], False, True)
                mm(poB, V4[:, NMT - 1, hB, :], prev_es[:, 512:1024], False, True)
                # drain PSUM fast (frees the po slots for the next head pair)
                denA = asmall.tile([1, 512], f32, name="denA", tag="denA")
                denB = asmall.tile([1, 512], f32, name="denB", tag="denB")
                nc.vector.tensor_copy(denA, poA[HD:VW, :])
                nc.vector.tensor_copy(denB, poB[HD:VW, :])
                posbP = asmall.tile([P, 512], f32, name="posbP", tag="posbP")
                nc.vector.tensor_copy(posbP[0:HD, :], poA[0:HD, :])
                nc.vector.tensor_copy(posbP[HD:P, :], poB[0:HD, :])
                rdA = asmall.tile([1, 512], f32r, name="rdA", tag="rdA")
                rdB = asmall.tile([1, 512], f32r, name="rdB", tag="rdB")
                pending = (j, cs, posbP, denA, denB, rdA, rdB)
            # flush at the chunk boundary: ch1's proj reads all ch0 OT2
            _, _, _, denA_, denB_, rdA_, rdB_ = pending
            with nc.allow_low_precision(reason="softmax recip"):
                nc.vector.reciprocal(rdA_, denA_)
                nc.vector.reciprocal(rdB_, denB_)
            emit_norm(pending[:3] + pending[5:])
            pending = None
            if ch == 0:
                wqp.release()
                wkp.release()
                hTp.release()
                res1p = tc.alloc_tile_pool(name="res1p", bufs=1)
                st_["res1"] = res1p.tile([P, NQT, D], bf16, name="res1")
                x2Tp = tc.alloc_tile_pool(name="x2Tp", bufs=1)
                st_["x2T"] = x2Tp.tile([P, DT, NQ], bf16, name="x2T")
                h1rp = tc.alloc_tile_pool(name="h1rp", bufs=1)
                st_["h1r"] = h1rp.tile([P, FC1_IN_CH1, 512], bf16, name="h1r")
                # first half of the fc2 weights streams during ch1
                f2lo = tc.alloc_tile_pool(name="f2lo", bufs=1)
                st_["w2lo"] = f2lo.tile([P, HIDT // 2, D], bf16, name="w2lo")
                for cc in range(2):
                    nc.gpsimd.dma_start(
                        out=st_["w2lo"][:, cc * 6:(cc + 1) * 6, :],
                        in_=aps["fc2_w"][cc * 6 * P:(cc + 1) * 6 * P, :]
                        .rearrange("(j p) d -> p j d", p=P),
                    )
    v4p.release()
    qtp.release()
    ktp.release()
    res1 = st_["res1"]
    x2T = st_["x2T"]
    h1r = st_["h1r"]

    # ---------------- tail ----------------
    w2lo = st_["w2lo"]
    f2hi = tc.alloc_tile_pool(name="f2hi", bufs=1)
    w2hi = f2hi.tile([P, HIDT // 2, D], bf16, name="w2hi")
    for cc in range(2):
        nc.gpsimd.dma_start(
            out=w2hi[:, cc * 6:(cc + 1) * 6, :],
            in_=aps["fc2_w"][(12 + cc * 6) * P:(12 + (cc + 1) * 6) * P, :]
            .rearrange("(j p) d -> p j d", p=P),
        )

    def w2sl(j, csl):
        if j < HIDT // 2:
            return w2lo[:, j, csl]
        return w2hi[:, j - HIDT // 2, csl]

    h1ap = tc.alloc_tile_pool(name="h1ap", bufs=1, side="right")
    h1a = h1ap.tile([P, FC1_IN_CH1, 512], bf16, name="h1a")
    h1bp = tc.alloc_tile_pool(name="h1bp", bufs=1, side="right")
    h1b = h1bp.tile([P, HIDT - FC1_IN_CH1, 512], bf16, name="h1b")
    h1cp = tc.alloc_tile_pool(name="h1cp", bufs=1, side="right")
    h1c = h1cp.tile([P, HIDT, 512], bf16, name="h1c")

    with tc.tile_pool(name="pTwork", bufs=2, side="right") as workT, \
         tc.tile_pool(name="pTsmall", bufs=2, side="right") as smallT, \
         tc.tile_pool(name="pTxr", bufs=2, side="right") as xrpT, \
         tc.tile_pool(name="pTpsQ", bufs=2, space="PSUM") as psQT, \
         tc.tile_pool(name="pTps2", bufs=2, space="PSUM") as ps2:

        ln2mvT = []

        def emit_projT(i):
            xr = xrpT.tile([P, D], f32, name="xrT", tag="xr")
            nc.sync.dma_start(out=xr, in_=x[i * P:(i + 1) * P, :])
            isl = slice(i * P, (i + 1) * P)
            psa = psQT.tile([P, 512], f32, name="pjaT", tag="cq")
            for j in range(JH):
                mm(psa, OT2[:, j, isl], pjw[:, j, 0:512], j == 0, j == JH - 1)
            psb = psQT.tile([P, 512], f32, name="pjbT", tag="cq")
            for j in range(JH):
                mm(psb[:, 0:256], OT2[:, j, isl], pjw[:, j, 512:768],
                   j == 0, j == JH - 1)
            r1 = res1[:, i, :]
            nc.vector.tensor_add(r1[:, 0:512], psa, xr[:, 0:512])
            nc.vector.tensor_add(r1[:, 512:768], psb[:, 0:256], xr[:, 512:768])
            nc.vector.tensor_tensor(r1, r1, pjbf, OP.add)
            stt = smallT.tile([P, 2, 6], f32, name="l2stT", tag=f"l2sT{i % 4}")
            for h2 in range(2):
                nc.vector.bn_stats(stt[:, h2, :], r1[:, h2 * 384:(h2 + 1) * 384])
            mv = smallT.tile([P, 2], f32, name="l2mvT", tag=f"l2mT{i % 4}")
            nc.vector.bn_aggr(mv, stt)
            ln2mvT.append(mv)

        # gelu for the ch1-deferred fc1 chains (first Gelu table load)
        nc.scalar.activation(h1a, h1r, AF.Gelu)

        # rest of fc1 chunk 0 first: independent of the attention-boundary
        # norm flush and of the LN2 chain, so the PE never head-blocks
        for hc in range(FC1_IN_CH1, HIDT):
            psf = psQT.tile([P, 512], f32, name="psfT", tag="cq")
            for t in range(DT):
                mm(psf, w1a[:, t, hc * P:(hc + 1) * P],
                   x2T[:, t, 0:512], t == 0, t == DT - 1)
            nc.scalar.activation(
                h1b[:, hc - FC1_IN_CH1, :], psf, AF.Gelu,
                bias=f1bc[:, hc:hc + 1],
            )

        for i in range(4, NQT):
            emit_projT(i)

        # LN2 for tiles 4-7 (same DVE Newton rsqrt)
        vre = smallT.tile([P, 4], f32, name="l2veT")
        vrT = smallT.tile([P, 4], f32, name="l2vrT")
        nmrT = smallT.tile([P, 4], f32, name="l2nmT")
        ttT = smallT.tile([P, 4], f32, name="l2ttT")
        for k in range(4):
            nc.vector.tensor_scalar_add(vre[:, k:k + 1], ln2mvT[k][:, 1:2], EPS)
        nc.vector.tensor_scalar(vrT, vre, -0.3346, 1.581, OP.mult, OP.add)
        for _ in range(4):
            nc.vector.scalar_tensor_tensor(ttT, vrT, 1.0, vrT, OP.mult, OP.mult)
            nc.vector.tensor_mul(ttT, ttT, vre)
            nc.vector.tensor_scalar(ttT, ttT, -0.5, 1.5, OP.mult, OP.add)
            nc.vector.tensor_mul(vrT, vrT, ttT)
        for k in range(4):
            nc.vector.tensor_scalar(
                nmrT[:, k:k + 1], ln2mvT[k][:, 0:1], vrT[:, k:k + 1], -1.0,
                OP.mult, OP.mult,
            )
        for k in range(4):
            i = 4 + k
            h2 = workT.tile([P, D], bf16, name="h2T")
            nc.vector.tensor_scalar(
                h2, res1[:, i, :], vrT[:, k:k + 1], nmrT[:, k:k + 1],
                OP.mult, OP.add,
            )
            for t in range(DT):
                ps = psQT.tile([P, P], f32, name="trpT", tag="cq")
                mm(ps, h2[:, t * P:(t + 1) * P], identb, True, True)
                if t % 2 == 0:
                    nc.scalar.activation(
                        x2T[:, t, i * P:(i + 1) * P], ps, AF.Identity,
                        bias=b2c[:, t:t + 1], scale=g2c[:, t:t + 1],
                    )
                else:
                    nc.vector.tensor_scalar(
                        x2T[:, t, i * P:(i + 1) * P], ps,
                        g2c[:, t:t + 1], b2c[:, t:t + 1], OP.mult, OP.add,
                    )

        # fc1 chunk 1 (direct gelu)
        for hc in range(HIDT):
            psf = psQT.tile([P, 512], f32, name="psfU", tag="cq")
            for t in range(DT):
                mm(psf, w1a[:, t, hc * P:(hc + 1) * P],
                   x2T[:, t, 512:1024], t == 0, t == DT - 1)
            nc.scalar.activation(
                h1c[:, hc, :], psf, AF.Gelu, bias=f1bc[:, hc:hc + 1]
            )

        def emit_fc2(i2, h1get):
            psq = ps2.tile([P, D], f32, name="psq", padded_shape=[P, 1024])
            qsl = slice((i2 % 4) * P, (i2 % 4 + 1) * P)
            for j in range(HIDT):
                mm(psq[:, 0:512], h1get(j, qsl), w2sl(j, slice(0, 512)),
                   j == 0, j == HIDT - 1)
            for j in range(HIDT):
                mm(psq[:, 512:768], h1get(j, qsl), w2sl(j, slice(512, 768)),
                   j == 0, j == HIDT - 1)
            ob = workT.tile([P, D], f32, name="outsb", tag="outsb", bufs=2)
            nc.vector.tensor_add(ob, res1[:, i2, :], psq)
            nc.sync.dma_start(out=out_ap[i2 * P:(i2 + 1) * P, :], in_=ob)

        def h1sl0(hc, qsl):
            if hc < FC1_IN_CH1:
                return h1a[:, hc, qsl]
            return h1b[:, hc - FC1_IN_CH1, qsl]

        for i2 in range(4):
            emit_fc2(i2, h1sl0)
        for i2 in range(4, NQT):
            emit_fc2(i2, lambda hc, qsl: h1c[:, hc, qsl])

    h1cp.release()
    h1bp.release()
    h1ap.release()
    f2hi.release()
    f2lo.release()
    h1rp.release()
    x2Tp.release()
    res1p.release()
    otp.release()
    f1wp.release()
    pjwp.release()
    consts.release()


def build_nc(hoist_waits=True):
    import concourse.bass as bass
    import concourse.tile as tile
    from concourse import mybir

    f32 = mybir.dt.float32
    nc = bass.Bass("TRN2", target_bir_lowering=False, debug=False)
    aps = {"x": nc.dram_tensor("x", [N, D], f32, kind="ExternalInput").ap()}
    shapes = {
        "ln1_g": [D], "ln1_b": [D], "qkv_w": [D, 3 * D],
        "proj_w": [D, D], "proj_b": [D], "ln2_g": [D], "ln2_b": [D],
        "fc1_w": [D, HID], "fc1_b": [HID], "fc2_w": [HID, D], "fc2_b": [D],
    }
    for name in INPUT_NAMES:
        aps[name] = nc.dram_tensor(name, shapes[name], f32, kind="ExternalInput").ap()
    out_ap = nc.dram_tensor("out", [NQ, D], f32, kind="ExternalOutput").ap()
    with tile.TileContext(nc) as tc:
        _encoder_body(tc, out_ap, aps)
    if hoist_waits:
        _hoist_matmul_waits(nc)
    return nc


def _hoist_matmul_waits(nc):
    """walrus's LW-path matmuls (transpose / fp32 / f32r self-loading) accept
    only one embedded sync-wait.  Tile can attach two (one per producer
    engine).  Hoist all-but-one onto a standalone InstEventSemaphore placed
    just before the matmul in the same engine stream."""
    from concourse import mybir

    skip = (
        mybir.InstEventSemaphore,
        mybir.InstUnconditionalBranch,
    )
    for f in nc.m.functions:
        for bb in f.blocks:
            out = []
            for ins in bb.instructions:
                si = getattr(ins, "sync_info", None)
                if (
                    si is not None
                    and si.on_wait
                    and len(si.on_wait) > 1
                    and not isinstance(ins, skip)
                ):
                    for k, wait in enumerate(si.on_wait[:-1]):
                        w = mybir.InstEventSemaphore(
                            name=f"{ins.name}-hoistwait{k}",
                            ins=[],
                            outs=[],
                        )
                        w.engine = ins.engine
                        w.sync_info = mybir.SyncInfo(on_wait=[wait], on_update=[])
                        out.append(w)
                    ins.sync_info = mybir.SyncInfo(
                        on_wait=[si.on_wait[-1]], on_update=list(si.on_update)
                    )
                out.append(ins)
            bb.instructions[:] = out


_NC_CACHE = {}


def make_in_maps(inputs):
    in_maps = []
    for c in range(8):
        b, s = c // 2, c % 2
        xb = np.asarray(inputs["x"][b], dtype=np.float32)
        xp = xb if s == 0 else np.ascontiguousarray(
            np.concatenate([xb[NQ:], xb[:NQ]], axis=0)
        )
        m = {"x": xp}
        for k in INPUT_NAMES:
            m[k] = np.asarray(inputs[k], dtype=np.float32)
        in_maps.append(m)
    return in_maps


def kernel(**inputs):
    from concourse import bass_utils

    if "nc" not in _NC_CACHE:
        _NC_CACHE["nc"] = build_nc()
    nc = _NC_CACHE["nc"]
    in_maps = make_in_maps(inputs)
    res = bass_utils.run_bass_kernel_spmd(nc, in_maps, core_ids=list(range(8)))
    out = np.empty((B, N, D), np.float32)
    for c in range(8):
        b, s = c // 2, c % 2
        out[b, s * NQ:(s + 1) * NQ] = res.results[c]["out"]
    return out


if __name__ == "__main__":
    rng = np.random.default_rng(0)
    fake = {
        "x": rng.standard_normal((B, N, D), dtype=np.float32),
        "ln1_g": np.ones(D, np.float32), "ln1_b": np.zeros(D, np.float32),
        "qkv_w": (rng.standard_normal((D, 3 * D)) / np.sqrt(D)).astype(np.float32),
        "proj_w": (rng.standard_normal((D, D)) / np.sqrt(D)).astype(np.float32),
        "proj_b": np.zeros(D, np.float32),
        "ln2_g": np.ones(D, np.float32), "ln2_b": np.zeros(D, np.float32),
        "fc1_w": (rng.standard_normal((D, HID)) / np.sqrt(D)).astype(np.float32),
        "fc1_b": np.zeros(HID, np.float32),
        "fc2_w": (rng.standard_normal((HID, D)) / np.sqrt(HID)).astype(np.float32),
        "fc2_b": np.zeros(D, np.float32),
    }
    out = kernel(**fake)
    print("kernel ran, out shape", out.shape)


# revision 40
# speedup vs baseline: 1.1705x; 1.1705x over previous
"""Trainium2 Bass kernel for a ViT-style EncoderBlock (v3).

Problem: B=4, N=2048, D=768, H=12 heads (hd=64), FFN hidden 3072, fp32.
  y = x + proj(attn(LN1(x))) ;  out = y + fc2(gelu(fc1(LN2(y))))

Sharding (8 cores, zero communication): core c handles batch b=c//2 and
query-half s=c%2 (1024 query rows).  Each core receives the full batch-b
sequence (2048 rows) with its own query rows permuted to the front, computes
K/V over all 2048 rows, attention/FFN for its 1024 rows, and returns its
[1024, 768] slice of the output.  Host reassembles.

v3 design notes (evolved from v2 = 546 us after trace analysis):
 - Phase A fuses LN1 and the V projection per 128-row x tile so the PE works
   through the LN phase instead of idling behind DVE stats; identity warm-up
   matmuls (some staggered on the x-tile DMAs) keep the HAM clock gate open.
 - Input DMAs split across rings: consts + QKV weights (per-d-column) on
   sync, x tiles + proj/fc1 weights on gpsimd; fc2 weights stream in 4
   chunks on both rings during the tail, ahead of the (reordered-last) fc2.
 - LN scale/bias applies alternate ACT / DVE (both read PSUM).
 - Attention runs ch(=query 512-chunk)-outer / head-pair-inner: after ch0,
   OT2[:, :, 0:512] is complete, so proj + residual + LN2 + 12 of 24 fc1
   chains for chunk 0 hide inside ch1's ACT-bound (softmax exp) window.
   All K/Q projection chains run inside ch0.
 - Softmax denominators: reciprocal_approx_fast (~5x cheaper on DVE); the
   two denominator broadcasts are column-tiled into one PSUM tile and the
   normalize multiply covers both heads at once.
 - LN2 uses a DVE-only Newton rsqrt so no ACT Sqrt table swap disturbs the
   Exp table during attention; ch1's fc1 chains defer their gelu (bias via
   DVE) until after attention (Gelu table loads once, in the tail).
 - Softmax skips max-subtraction (scores ~N(0,1) after 1/8 scale).

SBUF lifetime plan (stack allocator, LIFO per side):
  left:  consts, pjw, w1a, OT2, hT, wk, wq, wv, xball | phase-A pools
         (xball+wv released after phase A; hT/wk/wq after attention ch0;
          then res1/x2T/h1r; w2a at tail start)
  right: kT, qT, V4, attention pools (all released after attention),
         then h1a/h1b/h1c + tail pools
"""

import sys

if "/opt/trn_rl_repo" not in sys.path:
    sys.path.insert(0, "/opt/trn_rl_repo")

import numpy as np

B, N, D = 4, 2048, 768
H, HD = 12, 64
HID = 4 * D
NQ = N // 2  # query rows per core
SCALE = HD ** -0.5
EPS = 1e-5

P = 128
DT = D // P          # 6 d-tiles
NQT = NQ // P        # 8 query tiles
NMT = N // P         # 16 kv tiles
HIDT = HID // P      # 24 hidden tiles
VW = HD + 1          # 65: V plus ones column
JH = H // 2          # 6 head pairs
FC1_IN_CH1 = 12      # fc1 chunk-0 chains emitted inside attention ch1

INPUT_NAMES = (
    "ln1_g", "ln1_b", "qkv_w", "proj_w", "proj_b",
    "ln2_g", "ln2_b", "fc1_w", "fc1_b", "fc2_w", "fc2_b",
)


def _encoder_body(tc, out_ap, aps):
    import concourse.bass as bass
    from concourse import mybir
    from concourse.masks import make_identity

    nc = tc.nc
    f32 = mybir.dt.float32
    f32r = mybir.dt.float32r
    bf16 = mybir.dt.bfloat16
    AF = mybir.ActivationFunctionType
    OP = mybir.AluOpType

    def mm(psum, lhsT, rhs, start, stop):
        nc.tensor.matmul(psum, lhsT, rhs, start=start, stop=stop)

    x = aps["x"]

    # ---------------- pools, in LIFO release order ----------------
    consts = tc.alloc_tile_pool(name="consts", bufs=1)
    pjwp = tc.alloc_tile_pool(name="pjwp", bufs=1)
    f1wp = tc.alloc_tile_pool(name="f1w", bufs=1)
    otp = tc.alloc_tile_pool(name="otp", bufs=1)
    hTp = tc.alloc_tile_pool(name="hTp", bufs=1)
    wkp = tc.alloc_tile_pool(name="wkp", bufs=1)
    wqp = tc.alloc_tile_pool(name="wqp", bufs=1)
    wvp = tc.alloc_tile_pool(name="wvp", bufs=1)
    xbp = tc.alloc_tile_pool(name="xbp", bufs=1)
    ktp = tc.alloc_tile_pool(name="ktp", bufs=1, side="right")
    qtp = tc.alloc_tile_pool(name="qtp", bufs=1, side="right")
    v4p = tc.alloc_tile_pool(name="v4p", bufs=1, side="right")

    # ---------------- constants ----------------
    ident = consts.tile([P, P], f32, name="ident")
    make_identity(nc, ident)
    identb = consts.tile([P, P], bf16, name="identb")
    nc.vector.tensor_copy(identb, ident)
    g1c = consts.tile([P, DT], f32, name="g1c")
    b1c = consts.tile([P, DT], f32, name="b1c")
    g2c = consts.tile([P, DT], f32, name="g2c")
    b2c = consts.tile([P, DT], f32, name="b2c")
    f1bc = consts.tile([P, HIDT], f32, name="f1bc")
    pjbf = consts.tile([P, D], f32, name="pjbf")
    ones_f = consts.tile([P, HD], f32, name="ones_f")
    nc.vector.memset(ones_f, 1.0)
    ones64 = consts.tile([1, HD], f32r, name="ones64")
    nc.vector.tensor_copy(ones64, ones_f[0:1, :])

    # weights tiles
    pjw = pjwp.tile([P, JH, D], bf16, name="pjw")
    w1a = f1wp.tile([P, DT, HID], bf16, name="w1a")
    OT2 = otp.tile([P, JH, NQ], bf16, name="OT2")
    hT = hTp.tile([P, DT, N], bf16, name="hT")
    wk = wkp.tile([P, DT, D], bf16, name="wk")
    wq = wqp.tile([P, DT, D], bf16, name="wq")
    wv = wvp.tile([P, DT, D], bf16, name="wv")
    xball = xbp.tile([P, NMT, D], bf16, name="xball")
    kT = ktp.tile([P, DT, N], bf16, name="kT")
    qT = qtp.tile([P, DT, NQ], bf16, name="qT")
    V4 = v4p.tile([P, NMT, H, VW], bf16, name="V4")

    # ---------------- DMA issue (ring order matters) ----------------
    # sync ring (no cast allowed): small f32 consts.
    nc.sync.dma_start(out=g1c, in_=aps["ln1_g"].rearrange("(t p) -> p t", p=P))
    nc.sync.dma_start(out=b1c, in_=aps["ln1_b"].rearrange("(t p) -> p t", p=P))
    nc.sync.dma_start(out=g2c, in_=aps["ln2_g"].rearrange("(t p) -> p t", p=P))
    nc.sync.dma_start(out=b2c, in_=aps["ln2_b"].rearrange("(t p) -> p t", p=P))
    nc.sync.dma_start(out=f1bc, in_=aps["fc1_b"].rearrange("(t p) -> p t", p=P))

    # gpsimd ring (cast f32->bf16): first x tiles + V weights (phase A
    # needs them first), then K/Q weights, then proj + fc1 weights.
    for i in range(3):
        nc.gpsimd.dma_start(out=xball[:, i, :], in_=x[i * P:(i + 1) * P, :])
    for t in range(DT):
        nc.gpsimd.dma_start(
            out=wv[:, t, :], in_=aps["qkv_w"][t * P:(t + 1) * P, 2 * D:3 * D]
        )
    for i in range(3, 10):
        nc.gpsimd.dma_start(out=xball[:, i, :], in_=x[i * P:(i + 1) * P, :])
    for t in range(DT):
        nc.gpsimd.dma_start(
            out=wk[:, t, :], in_=aps["qkv_w"][t * P:(t + 1) * P, D:2 * D]
        )
    for i in range(10, NMT):
        nc.gpsimd.dma_start(out=xball[:, i, :], in_=x[i * P:(i + 1) * P, :])
    for t in range(DT):
        nc.gpsimd.dma_start(
            out=wq[:, t, :], in_=aps["qkv_w"][t * P:(t + 1) * P, 0:D]
        )
    nc.gpsimd.dma_start(out=pjw, in_=aps["proj_w"].rearrange("(j p) d -> p j d", p=P))
    nc.gpsimd.dma_start(out=w1a, in_=aps["fc1_w"].rearrange("(t p) c -> p t c", p=P))
    # pjbf = proj_b + fc2_b broadcast (both residual biases; see v2 notes),
    # built by two DMAs (second accumulates) — no engine op involved.
    # At the end of the gpsimd stream: needed only at the ch1 residuals.
    nc.gpsimd.dma_start(out=pjbf, in_=aps["proj_b"].partition_broadcast(P))
    nc.gpsimd.dma_start(
        out=pjbf, in_=aps["fc2_b"].partition_broadcast(P),
        accum_op=OP.add,
    )

    # ---------------- phase A : fused LN1 + V ----------------
    GRP = 2
    with tc.tile_pool(name="pAwork", bufs=2) as work, \
         tc.tile_pool(name="pAsmall", bufs=2) as small, \
         tc.tile_pool(name="pApsT", bufs=2, space="PSUM") as psT, \
         tc.tile_pool(name="pAboot", bufs=1, space="PSUM") as psBoot, \
         tc.tile_pool(name="pApsV", bufs=2, space="PSUM") as psV:
        # PE warm-up for the HAM clock gate
        wps = psT.tile([P, P], f32, name="warmps", tag="warm", bufs=1)
        for _ in range(48):
            mm(wps, identb, identb, True, True)
        # the V4 softmax-denominator ones column, all tiles in one memset
        nc.vector.memset(V4[:, :, :, HD:VW], 1.0)

        def boot_chain(kind, idx):
            """Bootstrap K/Q head-column-0 chains inside phase A so the
            attention loop starts with its operands ready."""
            ps = psBoot.tile([P, 512], f32, name="boot")
            w_ = wk if kind == "K" else wq
            for t in range(DT):
                mm(ps, w_[:, t, 0:P],
                   hT[:, t, idx * 512:(idx + 1) * 512], t == 0, t == DT - 1)
            if kind == "K":
                nc.vector.tensor_copy(kT[:, 0, idx * 512:(idx + 1) * 512], ps)
            else:
                nc.vector.tensor_copy(qT[:, 0, idx * 512:(idx + 1) * 512], ps)

        boots = {5: ("K", 0), 6: ("Q", 0), 7: ("K", 1)}
        for g in range(NMT // GRP):
            sts, mvs = [], []
            for k in range(GRP):
                i = g * GRP + k
                st = small.tile([P, 2, 6], f32, name=f"st{k}", tag=f"st{k}")
                for h2 in range(2):
                    nc.vector.bn_stats(
                        st[:, h2, :], xball[:, i, h2 * 384:(h2 + 1) * 384]
                    )
                sts.append(st)
            vre = small.tile([P, GRP], f32, name="vre", tag="vre")
            vr = small.tile([P, GRP], f32, name="vr", tag="vr")
            nmr = small.tile([P, GRP], f32, name="nmr", tag="nmr")
            sq = small.tile([P, GRP], f32, name="sq", tag="sq")
            r0 = small.tile([P, GRP], f32, name="r0", tag="r0")
            tt = small.tile([P, GRP], f32, name="tt", tag="tt")
            for k in range(GRP):
                mv = small.tile([P, 2], f32, name=f"mv{k}", tag=f"mv{k}")
                nc.vector.bn_aggr(mv, sts[k])
                mvs.append(mv)
                nc.vector.tensor_scalar_add(vre[:, k:k + 1], mv[:, 1:2], EPS)
            nc.scalar.sqrt(sq, vre)
            nc.vector.reciprocal(r0, sq)
            nc.vector.scalar_tensor_tensor(tt, r0, 1.0, r0, OP.mult, OP.mult)
            nc.vector.tensor_mul(tt, tt, vre)
            nc.vector.tensor_scalar(tt, tt, -0.5, 1.5, OP.mult, OP.add)
            nc.vector.tensor_mul(vr, r0, tt)
            for k in range(GRP):
                nc.vector.tensor_scalar(
                    nmr[:, k:k + 1], mvs[k][:, 0:1], vr[:, k:k + 1], -1.0,
                    OP.mult, OP.mult,
                )
            for k in range(GRP):
                i = g * GRP + k
                h = work.tile([P, D], bf16, name="hln")
                nc.vector.tensor_scalar(
                    h, xball[:, i, :], vr[:, k:k + 1], nmr[:, k:k + 1],
                    OP.mult, OP.add,
                )
                for t in range(DT):
                    ps = psT.tile([P, P], f32, name="trps")
                    mm(ps, h[:, t * P:(t + 1) * P], identb, True, True)
                    if t not in (2, 5):
                        nc.scalar.activation(
                            hT[:, t, i * P:(i + 1) * P], ps, AF.Identity,
                            bias=b1c[:, t:t + 1], scale=g1c[:, t:t + 1],
                        )
                    else:
                        nc.vector.tensor_scalar(
                            hT[:, t, i * P:(i + 1) * P], ps,
                            g1c[:, t:t + 1], b1c[:, t:t + 1], OP.mult, OP.add,
                        )
                psv = psV.tile([P, D], f32, name="vps", padded_shape=[P, 1024])
                for t in range(DT):
                    mm(psv[:, 0:512], hT[:, t, i * P:(i + 1) * P],
                       wv[:, t, 0:512], t == 0, t == DT - 1)
                for t in range(DT):
                    mm(psv[:, 512:768], hT[:, t, i * P:(i + 1) * P],
                       wv[:, t, 512:768], t == 0, t == DT - 1)
                nc.scalar.copy(
                    V4[:, i, 0:8, 0:HD],
                    psv[:, 0:512].rearrange("p (a b) -> p a b", a=8),
                )
                nc.scalar.copy(
                    V4[:, i, 8:12, 0:HD],
                    psv[:, 512:768].rearrange("p (a b) -> p a b", a=4),
                )
            if g in boots:
                boot_chain(*boots[g])
        boot_chain("K", 2)
        boot_chain("K", 3)
    xbp.release()
    wvp.release()

    # ---------------- phase B : attention (ch outer, j inner) -------------
    st_ = {}  # late-bound tiles (res1/x2T/h1r created after ch0)

    with tc.tile_pool(name="a_es", bufs=3, side="right") as esp, \
         tc.tile_pool(name="a_small", bufs=1, side="right") as asmall, \
         tc.tile_pool(name="a_ln2", bufs=2, side="right") as lsm, \
         tc.tile_pool(name="a_xr", bufs=2, side="right") as xrp, \
         tc.tile_pool(name="a_psS", bufs=2, space="PSUM") as psS, \
         tc.tile_pool(name="a_psO", bufs=2, space="PSUM") as psO, \
         tc.tile_pool(name="a_psQ", bufs=2, space="PSUM") as psQ:

        def emit_kq_chain(kind, c, idx):
            ps = psQ.tile([P, 512], f32, name="cq", tag="cq")
            w_ = wk if kind == "K" else wq
            for t in range(DT):
                mm(ps, w_[:, t, c * P:(c + 1) * P],
                   hT[:, t, idx * 512:(idx + 1) * 512], t == 0, t == DT - 1)
            if kind == "K":
                nc.vector.tensor_copy(kT[:, c, idx * 512:(idx + 1) * 512], ps)
            else:
                nc.vector.tensor_copy(qT[:, c, idx * 512:(idx + 1) * 512], ps)

        def emit_norm(stt):
            """Column-tiled broadcast of the two reciprocal denominator rows
            over the 64 head rows each, then one normalize multiply."""
            j, cs, posbP, rdA, rdB = stt
            rbA = psQ.tile([HD, 512], f32, name="rbA", tag="cq")
            mm(rbA, ones64, rdA, True, True)
            nc.vector.tensor_tensor(OT2[0:HD, j, cs], posbP[0:HD, :], rbA, OP.mult)
            rbB = psQ.tile([HD, 512], f32, name="rbB", tag="cq")
            mm(rbB, ones64, rdB, True, True)
            nc.vector.tensor_tensor(OT2[HD:P, j, cs], posbP[HD:P, :], rbB, OP.mult)

        ln2mv = []

        def emit_xr_dma(i):
            xr = xrp.tile([P, D], f32, name="xr", tag="xr")
            nc.sync.dma_start(out=xr, in_=x[i * P:(i + 1) * P, :])
            return xr

        def emit_proj(i, xr):
            isl = slice(i * P, (i + 1) * P)
            psa = psQ.tile([P, 512], f32, name="pja", tag="cq")
            for j in range(JH):
                mm(psa, OT2[:, j, isl], pjw[:, j, 0:512], j == 0, j == JH - 1)
            psb = psQ.tile([P, 512], f32, name="pjb2", tag="cq")
            for j in range(JH):
                mm(psb[:, 0:256], OT2[:, j, isl], pjw[:, j, 512:768],
                   j == 0, j == JH - 1)
            r1 = st_["res1"][:, i, :]
            nc.vector.tensor_add(r1[:, 0:512], psa, xr[:, 0:512])
            nc.vector.tensor_add(r1[:, 512:768], psb[:, 0:256], xr[:, 512:768])
            nc.vector.tensor_tensor(r1, r1, pjbf, OP.add)
            stt = lsm.tile([P, 2, 6], f32, name="l2st", tag=f"l2st{i % 4}")
            for h2 in range(2):
                nc.vector.bn_stats(stt[:, h2, :], r1[:, h2 * 384:(h2 + 1) * 384])
            mv = lsm.tile([P, 2], f32, name="l2mv", tag=f"l2mv{i % 4}")
            nc.vector.bn_aggr(mv, stt)
            ln2mv.append(mv)

        def emit_ln2_batch(base, nt):
            """DVE-only Newton rsqrt for nt tiles (keeps Exp table loaded)."""
            vre = lsm.tile([P, nt], f32, name="l2ve", tag="l2ve")
            vr = lsm.tile([P, nt], f32, name="l2vr", tag="l2vr")
            nmr = lsm.tile([P, nt], f32, name="l2nm", tag="l2nm")
            tt = lsm.tile([P, nt], f32, name="l2tt", tag="l2tt")
            for k in range(nt):
                nc.vector.tensor_scalar_add(
                    vre[:, k:k + 1], ln2mv[base + k][:, 1:2], EPS
                )
            # minimax line seed for 1/sqrt on [0.5, 3.0] + 4 Newton steps
            nc.vector.tensor_scalar(vr, vre, -0.3346, 1.581, OP.mult, OP.add)
            for _ in range(4):
                nc.vector.scalar_tensor_tensor(tt, vr, 1.0, vr, OP.mult, OP.mult)
                nc.vector.tensor_mul(tt, tt, vre)
                nc.vector.tensor_scalar(tt, tt, -0.5, 1.5, OP.mult, OP.add)
                nc.vector.tensor_mul(vr, vr, tt)
            for k in range(nt):
                nc.vector.tensor_scalar(
                    nmr[:, k:k + 1], ln2mv[base + k][:, 0:1], vr[:, k:k + 1],
                    -1.0, OP.mult, OP.mult,
                )
            return vr, nmr

        h2cache = {}

        def emit_trans2(i, vr, nmr, k, tlist):
            if i not in h2cache:
                h2 = lsm.tile([P, D], bf16, name="h2", tag=f"h2{i % 2}")
                nc.vector.tensor_scalar(
                    h2, st_["res1"][:, i, :], vr[:, k:k + 1], nmr[:, k:k + 1],
                    OP.mult, OP.add,
                )
                h2cache[i] = h2
            h2 = h2cache[i]
            for t in tlist:
                ps = psQ.tile([P, P], f32, name="trp2", tag="cq")
                mm(ps, h2[:, t * P:(t + 1) * P], identb, True, True)
                if t % 2 == 0:
                    nc.scalar.activation(
                        st_["x2T"][:, t, i * P:(i + 1) * P], ps, AF.Identity,
                        bias=b2c[:, t:t + 1], scale=g2c[:, t:t + 1],
                    )
                else:
                    nc.vector.tensor_scalar(
                        st_["x2T"][:, t, i * P:(i + 1) * P], ps,
                        g2c[:, t:t + 1], b2c[:, t:t + 1], OP.mult, OP.add,
                    )

        def emit_fc1_deferred(hc):
            psf = psQ.tile([P, 512], f32, name="psf", tag="cq")
            for t in range(DT):
                mm(psf, w1a[:, t, hc * P:(hc + 1) * P],
                   st_["x2T"][:, t, 0:512], t == 0, t == DT - 1)
            nc.vector.tensor_scalar_add(
                st_["h1r"][:, hc, :], psf, f1bc[:, hc:hc + 1]
            )

        # ---------------- attention main loops ----------------
        ln2vn = []
        pending = None
        for ch in range(2):
            cs = slice(ch * 512, (ch + 1) * 512)
            for j in range(JH):
                hA, hB = 2 * j, 2 * j + 1
                kTa, kTb = kT[0:HD, j, :], kT[HD:P, j, :]
                qTa, qTb = qT[0:HD, j, cs], qT[HD:P, j, cs]
                witems = {}
                if ch == 0:
                    if j + 1 < JH:
                        witems = {
                            2: lambda c=j + 1: emit_kq_chain("K", c, 0),
                            4: lambda c=j + 1: emit_kq_chain("K", c, 1),
                            6: lambda c=j + 1: emit_kq_chain("K", c, 2),
                            8: lambda c=j + 1: emit_kq_chain("K", c, 3),
                            13: lambda c=j + 1: emit_kq_chain("Q", c, 0),
                            15: lambda c=j: emit_kq_chain("Q", c, 1),
                        }
                    else:
                        witems = {2: lambda c=j: emit_kq_chain("Q", c, 1)}
                else:
                    if j == 0:
                        # slots after mt11: the carried-over ch0-j5 norm
                        # lands at mt11, and proj reads all of ch0's OT2
                        witems = {
                            13: lambda: emit_proj(0, emit_xr_dma(0)),
                            15: lambda: emit_proj(1, emit_xr_dma(1)),
                        }
                    elif j == 1:
                        witems = {
                            2: lambda: emit_proj(2, emit_xr_dma(2)),
                            6: lambda: emit_proj(3, emit_xr_dma(3)),
                        }
                    elif j == 2:
                        def _ln2_head():
                            ln2vn.append(emit_ln2_batch(0, 4))
                            h2cache.clear()
                            emit_trans2(0, *ln2vn[0], 0, [0, 1, 2])
                        witems = {
                            2: _ln2_head,
                            4: lambda: emit_trans2(0, *ln2vn[0], 0, [3, 4, 5]),
                            6: lambda: emit_trans2(1, *ln2vn[0], 1, [0, 1, 2]),
                            8: lambda: emit_trans2(1, *ln2vn[0], 1, [3, 4, 5]),
                            13: lambda: emit_trans2(2, *ln2vn[0], 2, [0, 1, 2]),
                            15: lambda: emit_trans2(2, *ln2vn[0], 2, [3, 4, 5]),
                        }
                    elif j == 3:
                        witems = {
                            2: lambda: emit_trans2(3, *ln2vn[0], 3, [0, 1, 2]),
                            4: lambda: emit_trans2(3, *ln2vn[0], 3, [3, 4, 5]),
                            6: lambda: emit_fc1_deferred(0),
                            8: lambda: emit_fc1_deferred(1),
                            13: lambda: emit_fc1_deferred(2),
                            15: lambda: emit_fc1_deferred(3),
                        }
                    elif j == 4:
                        witems = {
                            2: lambda: emit_fc1_deferred(4),
                            6: lambda: emit_fc1_deferred(5),
                            8: lambda: emit_fc1_deferred(6),
                            13: lambda: emit_fc1_deferred(7),
                        }
                    elif j == 5:
                        witems = {
                            2: lambda: emit_fc1_deferred(8),
                            6: lambda: emit_fc1_deferred(9),
                            8: lambda: emit_fc1_deferred(10),
                            13: lambda: emit_fc1_deferred(11),
                        }
                poA = psO.tile([VW, 512], f32, name="poA", tag="po")
                poB = psO.tile([VW, 512], f32, name="poB", tag="po")
                # software-pipelined: scores(mt) and exp(mt) issue before
                # attnV(mt-1), so the PE always has scores work in flight
                # while the ACT exp runs
                prev_es = None
                for mt in range(NMT):
                    msl = slice(mt * P, (mt + 1) * P)
                    ps = psS.tile([P, 1024], f32, name="sps")
                    mm(ps[:, 0:512], kTa[:, msl], qTa, True, True)
                    mm(ps[:, 512:1024], kTb[:, msl], qTb, True, True)
                    es = esp.tile([P, 1024], bf16, name="es")
                    nc.scalar.activation(es, ps, AF.Exp, scale=SCALE)
                    if prev_es is not None:
                        mm(poA, V4[:, mt - 1, hA, :], prev_es[:, 0:512],
                           mt - 1 == 0, False)
                        mm(poB, V4[:, mt - 1, hB, :], prev_es[:, 512:1024],
                           mt - 1 == 0, False)
                    prev_es = es
                    if mt == 5 and pending is not None:
                        # previous pair's reciprocals, off the drain path so
                        # they never sit ahead of this pair's PSUM casts
                        _, _, _, denA_, denB_, rdA_, rdB_ = pending
                        with nc.allow_low_precision(reason="softmax recip"):
                            nc.vector.reciprocal(rdA_, denA_)
                            nc.vector.reciprocal(rdB_, denB_)
                    if mt == 11 and pending is not None:
                        emit_norm(pending[:3] + pending[5:])
                        pending = None
                    w = witems.get(mt)
                    if w is not None:
                        w()
                mm(poA, V4[:, NMT - 1, hA, :], prev_es[:, 0:512], False, True)
                mm(poB, V4[:, NMT - 1, hB, :], prev_es[:, 512:1024], False, True)
                # drain PSUM fast (frees the po slots for the next head pair)
                denA = asmall.tile([1, 512], f32, name="denA", tag="denA")
                denB = asmall.tile([1, 512], f32, name="denB", tag="denB")
                nc.vector.tensor_copy(denA, poA[HD:VW, :])
                nc.vector.tensor_copy(denB, poB[HD:VW, :])
                posbP = asmall.tile([P, 512], f32, name="posbP", tag="posbP")
                nc.vector.tensor_copy(posbP[0:HD, :], poA[0:HD, :])
                nc.vector.tensor_copy(posbP[HD:P, :], poB[0:HD, :])
                rdA = asmall.tile([1, 512], f32r, name="rdA", tag="rdA")
                rdB = asmall.tile([1, 512], f32r, name="rdB", tag="rdB")
                pending = (j, cs, posbP, denA, denB, rdA, rdB)
            # the pending norm carries across the ch boundary through the
            # normal pipelined path (recips at ch1-j0 mt5, norm at mt11)
            if ch == 0:
                wqp.release()
                wkp.release()
                hTp.release()
                res1p = tc.alloc_tile_pool(name="res1p", bufs=1)
                st_["res1"] = res1p.tile([P, NQT, D], bf16, name="res1")
                x2Tp = tc.alloc_tile_pool(name="x2Tp", bufs=1)
                st_["x2T"] = x2Tp.tile([P, DT, NQ], bf16, name="x2T")
                h1rp = tc.alloc_tile_pool(name="h1rp", bufs=1)
                st_["h1r"] = h1rp.tile([P, FC1_IN_CH1, 512], bf16, name="h1r")
                # first half of the fc2 weights streams during ch1
                f2lo = tc.alloc_tile_pool(name="f2lo", bufs=1)
                st_["w2lo"] = f2lo.tile([P, HIDT // 2, D], bf16, name="w2lo")
                for cc in range(2):
                    nc.gpsimd.dma_start(
                        out=st_["w2lo"][:, cc * 6:(cc + 1) * 6, :],
                        in_=aps["fc2_w"][cc * 6 * P:(cc + 1) * 6 * P, :]
                        .rearrange("(j p) d -> p j d", p=P),
                    )
        # final flush for ch1-j5 (the tail starts with fc1 work that does
        # not touch OT2, so this no longer head-blocks the PE)
        _, _, _, denA_, denB_, rdA_, rdB_ = pending
        with nc.allow_low_precision(reason="softmax recip"):
            nc.vector.reciprocal(rdA_, denA_)
            nc.vector.reciprocal(rdB_, denB_)
        emit_norm(pending[:3] + pending[5:])
        pending = None
    v4p.release()
    qtp.release()
    ktp.release()
    res1 = st_["res1"]
    x2T = st_["x2T"]
    h1r = st_["h1r"]

    # ---------------- tail ----------------
    w2lo = st_["w2lo"]
    f2hi = tc.alloc_tile_pool(name="f2hi", bufs=1)
    w2hi = f2hi.tile([P, HIDT // 2, D], bf16, name="w2hi")
    for cc in range(2):
        nc.gpsimd.dma_start(
            out=w2hi[:, cc * 6:(cc + 1) * 6, :],
            in_=aps["fc2_w"][(12 + cc * 6) * P:(12 + (cc + 1) * 6) * P, :]
            .rearrange("(j p) d -> p j d", p=P),
        )

    def w2sl(j, csl):
        if j < HIDT // 2:
            return w2lo[:, j, csl]
        return w2hi[:, j - HIDT // 2, csl]

    h1ap = tc.alloc_tile_pool(name="h1ap", bufs=1, side="right")
    h1a = h1ap.tile([P, FC1_IN_CH1, 512], bf16, name="h1a")
    h1bp = tc.alloc_tile_pool(name="h1bp", bufs=1, side="right")
    h1b = h1bp.tile([P, HIDT - FC1_IN_CH1, 512], bf16, name="h1b")
    h1cp = tc.alloc_tile_pool(name="h1cp", bufs=1, side="right")
    h1c = h1cp.tile([P, HIDT, 512], bf16, name="h1c")

    with tc.tile_pool(name="pTwork", bufs=2, side="right") as workT, \
         tc.tile_pool(name="pTsmall", bufs=2, side="right") as smallT, \
         tc.tile_pool(name="pTxr", bufs=2, side="right") as xrpT, \
         tc.tile_pool(name="pTpsQ", bufs=2, space="PSUM") as psQT, \
         tc.tile_pool(name="pTps2", bufs=2, space="PSUM") as ps2:

        ln2mvT = []

        def emit_projT(i):
            xr = xrpT.tile([P, D], f32, name="xrT", tag="xr")
            nc.sync.dma_start(out=xr, in_=x[i * P:(i + 1) * P, :])
            isl = slice(i * P, (i + 1) * P)
            psa = psQT.tile([P, 512], f32, name="pjaT", tag="cq")
            for j in range(JH):
                mm(psa, OT2[:, j, isl], pjw[:, j, 0:512], j == 0, j == JH - 1)
            psb = psQT.tile([P, 512], f32, name="pjbT", tag="cq")
            for j in range(JH):
                mm(psb[:, 0:256], OT2[:, j, isl], pjw[:, j, 512:768],
                   j == 0, j == JH - 1)
            r1 = res1[:, i, :]
            nc.vector.tensor_add(r1[:, 0:512], psa, xr[:, 0:512])
            nc.vector.tensor_add(r1[:, 512:768], psb[:, 0:256], xr[:, 512:768])
            nc.vector.tensor_tensor(r1, r1, pjbf, OP.add)
            stt = smallT.tile([P, 2, 6], f32, name="l2stT", tag=f"l2sT{i % 4}")
            for h2 in range(2):
                nc.vector.bn_stats(stt[:, h2, :], r1[:, h2 * 384:(h2 + 1) * 384])
            mv = smallT.tile([P, 2], f32, name="l2mvT", tag=f"l2mT{i % 4}")
            nc.vector.bn_aggr(mv, stt)
            ln2mvT.append(mv)

        # gelu for the ch1-deferred fc1 chains (first Gelu table load)
        nc.scalar.activation(h1a, h1r, AF.Gelu)

        # rest of fc1 chunk 0 first: independent of the attention-boundary
        # norm flush and of the LN2 chain, so the PE never head-blocks
        for hc in range(FC1_IN_CH1, HIDT):
            psf = psQT.tile([P, 512], f32, name="psfT", tag="cq")
            for t in range(DT):
                mm(psf, w1a[:, t, hc * P:(hc + 1) * P],
                   x2T[:, t, 0:512], t == 0, t == DT - 1)
            nc.scalar.activation(
                h1b[:, hc - FC1_IN_CH1, :], psf, AF.Gelu,
                bias=f1bc[:, hc:hc + 1],
            )

        for i in range(4, NQT):
            emit_projT(i)

        # LN2 for tiles 4-7 (same DVE Newton rsqrt)
        vre = smallT.tile([P, 4], f32, name="l2veT")
        vrT = smallT.tile([P, 4], f32, name="l2vrT")
        nmrT = smallT.tile([P, 4], f32, name="l2nmT")
        ttT = smallT.tile([P, 4], f32, name="l2ttT")
        for k in range(4):
            nc.vector.tensor_scalar_add(vre[:, k:k + 1], ln2mvT[k][:, 1:2], EPS)
        nc.vector.tensor_scalar(vrT, vre, -0.3346, 1.581, OP.mult, OP.add)
        for _ in range(4):
            nc.vector.scalar_tensor_tensor(ttT, vrT, 1.0, vrT, OP.mult, OP.mult)
            nc.vector.tensor_mul(ttT, ttT, vre)
            nc.vector.tensor_scalar(ttT, ttT, -0.5, 1.5, OP.mult, OP.add)
            nc.vector.tensor_mul(vrT, vrT, ttT)
        for k in range(4):
            nc.vector.tensor_scalar(
                nmrT[:, k:k + 1], ln2mvT[k][:, 0:1], vrT[:, k:k + 1], -1.0,
                OP.mult, OP.mult,
            )
        for k in range(4):
            i = 4 + k
            h2 = workT.tile([P, D], bf16, name="h2T")
            nc.vector.tensor_scalar(
                h2, res1[:, i, :], vrT[:, k:k + 1], nmrT[:, k:k + 1],
                OP.mult, OP.add,
            )
            for t in range(DT):
                ps = psQT.tile([P, P], f32, name="trpT", tag="cq")
                mm(ps, h2[:, t * P:(t + 1) * P], identb, True, True)
                if t % 2 == 0:
                    nc.scalar.activation(
                        x2T[:, t, i * P:(i + 1) * P], ps, AF.Identity,
                        bias=b2c[:, t:t + 1], scale=g2c[:, t:t + 1],
                    )
                else:
                    nc.vector.tensor_scalar(
                        x2T[:, t, i * P:(i + 1) * P], ps,
                        g2c[:, t:t + 1], b2c[:, t:t + 1], OP.mult, OP.add,
                    )

        # fc1 chunk 1 (direct gelu)
        for hc in range(HIDT):
            psf = psQT.tile([P, 512], f32, name="psfU", tag="cq")
            for t in range(DT):
                mm(psf, w1a[:, t, hc * P:(hc + 1) * P],
                   x2T[:, t, 512:1024], t == 0, t == DT - 1)
            nc.scalar.activation(
                h1c[:, hc, :], psf, AF.Gelu, bias=f1bc[:, hc:hc + 1]
            )

        def emit_fc2(i2, h1get):
            psq = ps2.tile([P, D], f32, name="psq", padded_shape=[P, 1024])
            qsl = slice((i2 % 4) * P, (i2 % 4 + 1) * P)
            for j in range(HIDT):
                mm(psq[:, 0:512], h1get(j, qsl), w2sl(j, slice(0, 512)),
                   j == 0, j == HIDT - 1)
            for j in range(HIDT):
                mm(psq[:, 512:768], h1get(j, qsl), w2sl(j, slice(512, 768)),
                   j == 0, j == HIDT - 1)
            ob = workT.tile([P, D], f32, name="outsb", tag="outsb", bufs=2)
            nc.vector.tensor_add(ob, res1[:, i2, :], psq)
            nc.sync.dma_start(out=out_ap[i2 * P:(i2 + 1) * P, :], in_=ob)

        def h1sl0(hc, qsl):
            if hc < FC1_IN_CH1:
                return h1a[:, hc, qsl]
            return h1b[:, hc - FC1_IN_CH1, qsl]

        for i2 in range(4):
            emit_fc2(i2, h1sl0)
        for i2 in range(4, NQT):
            emit_fc2(i2, lambda hc, qsl: h1c[:, hc, qsl])

    h1cp.release()
    h1bp.release()
    h1ap.release()
    f2hi.release()
    f2lo.release()
    h1rp.release()
    x2Tp.release()
    res1p.release()
    otp.release()
    f1wp.release()
    pjwp.release()
    consts.release()


def build_nc(hoist_waits=True):
    import concourse.bass as bass
    import concourse.tile as tile
    from concourse import mybir

    f32 = mybir.dt.float32
    nc = bass.Bass("TRN2", target_bir_lowering=False, debug=False)
    aps = {"x": nc.dram_tensor("x", [N, D], f32, kind="ExternalInput").ap()}
    shapes = {
        "ln1_g": [D], "ln1_b": [D], "qkv_w": [D, 3 * D],
        "proj_w": [D, D], "proj_b": [D], "ln2_g": [D], "ln2_b": [D],
        "fc1_w": [D, HID], "fc1_b": [HID], "fc2_w": [HID, D], "fc2_b": [D],
    }
    for name in INPUT_NAMES:
        aps[name] = nc.dram_tensor(name, shapes[name], f32, kind="ExternalInput").ap()
    out_ap = nc.dram_tensor("out", [NQ, D], f32, kind="ExternalOutput").ap()
    with tile.TileContext(nc) as tc:
        _encoder_body(tc, out_ap, aps)
    if hoist_waits:
        _hoist_matmul_waits(nc)
    return nc


def _hoist_matmul_waits(nc):
    """walrus's LW-path matmuls (transpose / fp32 / f32r self-loading) accept
    only one embedded sync-wait.  Tile can attach two (one per producer
    engine).  Hoist all-but-one onto a standalone InstEventSemaphore placed
    just before the matmul in the same engine stream."""
    from concourse import mybir

    skip = (
        mybir.InstEventSemaphore,
        mybir.InstUnconditionalBranch,
    )
    for f in nc.m.functions:
        for bb in f.blocks:
            out = []
            for ins in bb.instructions:
                si = getattr(ins, "sync_info", None)
                if (
                    si is not None
                    and si.on_wait
                    and len(si.on_wait) > 1
                    and not isinstance(ins, skip)
                ):
                    for k, wait in enumerate(si.on_wait[:-1]):
                        w = mybir.InstEventSemaphore(
                            name=f"{ins.name}-hoistwait{k}",
                            ins=[],
                            outs=[],
                        )
                        w.engine = ins.engine
                        w.sync_info = mybir.SyncInfo(on_wait=[wait], on_update=[])
                        out.append(w)
                    ins.sync_info = mybir.SyncInfo(
                        on_wait=[si.on_wait[-1]], on_update=list(si.on_update)
                    )
                out.append(ins)
            bb.instructions[:] = out


_NC_CACHE = {}


def make_in_maps(inputs):
    in_maps = []
    for c in range(8):
        b, s = c // 2, c % 2
        xb = np.asarray(inputs["x"][b], dtype=np.float32)
        xp = xb if s == 0 else np.ascontiguousarray(
            np.concatenate([xb[NQ:], xb[:NQ]], axis=0)
        )
        m = {"x": xp}
        for k in INPUT_NAMES:
            m[k] = np.asarray(inputs[k], dtype=np.float32)
        in_maps.append(m)
    return in_maps


def kernel(**inputs):
    from concourse import bass_utils

    if "nc" not in _NC_CACHE:
        _NC_CACHE["nc"] = build_nc()
    nc = _NC_CACHE["nc"]
    in_maps = make_in_maps(inputs)
    res = bass_utils.run_bass_kernel_spmd(nc, in_maps, core_ids=list(range(8)))
    out = np.empty((B, N, D), np.float32)
    for c in range(8):
        b, s = c // 2, c % 2
        out[b, s * NQ:(s + 1) * NQ] = res.results[c]["out"]
    return out


if __name__ == "__main__":
    rng = np.random.default_rng(0)
    fake = {
        "x": rng.standard_normal((B, N, D), dtype=np.float32),
        "ln1_g": np.ones(D, np.float32), "ln1_b": np.zeros(D, np.float32),
        "qkv_w": (rng.standard_normal((D, 3 * D)) / np.sqrt(D)).astype(np.float32),
        "proj_w": (rng.standard_normal((D, D)) / np.sqrt(D)).astype(np.float32),
        "proj_b": np.zeros(D, np.float32),
        "ln2_g": np.ones(D, np.float32), "ln2_b": np.zeros(D, np.float32),
        "fc1_w": (rng.standard_normal((D, HID)) / np.sqrt(D)).astype(np.float32),
        "fc1_b": np.zeros(HID, np.float32),
        "fc2_w": (rng.standard_normal((HID, D)) / np.sqrt(HID)).astype(np.float32),
        "fc2_b": np.zeros(D, np.float32),
    }
    out = kernel(**fake)
    print("kernel ran, out shape", out.shape)
